# revision 2
# baseline (speedup 1.0000x reference)
"""FP8Linear Trainium2 kernel.

Computes out = x @ (dequant(weight_fp8) * scale_w)^T + bias for
x: (4, 8192, 2048) bf16, weight_fp8: (2048, 2048) fp8_e4m3fn, scale_w: scalar f32,
bias: (2048,) bf16  ->  out: (4, 8192, 2048) bf16.

Strategy: data-parallel over rows. x flattens to (32768, 2048); each of the 8
NeuronCores gets a contiguous 4096-row slice, the (small, 4MB) weight is
replicated, and each core computes its slice of the output independently — no
collectives. Per core this is compute-bound: ~34 GFLOP of bf16 matmul (~437us
at peak) vs ~38MB of DMA traffic.

fp8 handling: the e4m3fn bytes are shipped as uint8 (the OCP format is NOT
TRN's FP8_EXP4 — values in (240, 448] would decode as NaN/Inf on TRN), and
dequantized on-device with integer ops:
    bf16_bits = ((b << 4) & 0x07F0) | ((b << 8) & 0x8000)
which places exponent e at the bf16 exponent field, i.e. value = fp8_value *
2^-120 exactly (incl. subnormals). One ACT copy with scale = 2^120 *
bf16(scale_w) then produces weights bit-identical to the reference's
  (fp8 -> bf16) * bf16(scale_w).
"""

import sys

sys.path.insert(0, "/opt/trn_rl_repo")

import numpy as np
import ml_dtypes

import concourse.bass as bass
import concourse.mybir as mybir
import concourse.tile as tile
from concourse import bacc
from concourse.bass_utils import run_bass_kernel_spmd

P = 128
N_CORES = 8

# Full problem dims (hardcoded per the contract).
B, S, IN, OUT = 4, 8192, 2048, 2048
M_TOTAL = B * S
M_CORE = M_TOTAL // N_CORES


def emit_fp8linear(tc, out, x, w_t, scale, bias_row, M, IN_, OUT_, MB, NO):
    """Emit the per-core program.

    out:   [M, OUT_]  bf16 DRAM (ExternalOutput)
    x:     [M, IN_]   bf16 DRAM
    w_t:   [IN_, OUT_] uint8 DRAM — fp8e4m3fn bytes of W^T (d-major)
    scale: [1, 1]     f32 DRAM
    bias_row: [1, OUT_] bf16 DRAM
    MB: m-block rows (DMA-transpose granularity), NO: matmul moving free dim.
    """
    nc = tc.nc
    DT = IN_ // P  # contraction (d) tiles
    OG = OUT_ // NO  # output column groups
    NMB = M // MB  # m blocks
    MS = MB // P  # 128-row subtiles per m block
    dt_bf16 = mybir.dt.bfloat16
    dt_u8 = mybir.dt.uint8
    dt_u16 = mybir.dt.uint16
    dt_f32 = mybir.dt.float32
    Alu = mybir.AluOpType

    with (
        tc.tile_pool(name="const", bufs=1) as const,
        tc.tile_pool(name="wpool", bufs=1) as wpool,
        tc.tile_pool(name="wstage", bufs=2) as wstage,
        tc.tile_pool(name="xT", bufs=2 * DT) as xp,
        tc.tile_pool(name="psum", bufs=8, space="PSUM") as pp,
        tc.tile_pool(name="obuf", bufs=4) as op,
    ):
        # ---- constants ----
        bias_t = const.tile([P, OUT_], dt_bf16)
        nc.sync.dma_start(bias_t[:], bias_row.to_broadcast((P, OUT_)))

        s_raw = const.tile([P, 1], dt_f32)
        nc.sync.dma_start(s_raw[:], scale.to_broadcast((P, 1)))
        # round scale to bf16 (reference multiplies by bf16(scale_w)), then
        # fold in the 2^120 exponent-offset of the integer decode below.
        s_bf = const.tile([P, 1], dt_bf16)
        nc.vector.tensor_copy(s_bf[:], s_raw[:])
        s_eff = const.tile([P, 1], dt_f32)
        nc.vector.tensor_scalar(
            out=s_eff[:], in0=s_bf[:], scalar1=float(2.0**120), scalar2=None,
            op0=Alu.mult,
        )

        # ---- weight decode: wT[dt] = bf16 dequant of W^T[dt*P:(dt+1)*P, :] ----
        wts = []
        for dt in range(DT):
            wu8 = wstage.tile([P, OUT_], dt_u8, tag="wu8")
            nc.sync.dma_start(wu8[:], w_t[dt * P:(dt + 1) * P, :])
            wu16 = wstage.tile([P, OUT_], dt_u16, tag="wu16")
            nc.vector.tensor_copy(wu16[:], wu8[:])  # zero-extend u8 -> u16
            t1 = wstage.tile([P, OUT_], dt_u16, tag="t1")
            nc.vector.tensor_scalar(
                out=t1[:], in0=wu16[:], scalar1=4, scalar2=0x07F0,
                op0=Alu.logical_shift_left, op1=Alu.bitwise_and,
            )
            t2 = wstage.tile([P, OUT_], dt_u16, tag="t2")
            nc.vector.tensor_scalar(
                out=t2[:], in0=wu16[:], scalar1=8, scalar2=0x8000,
                op0=Alu.logical_shift_left, op1=Alu.bitwise_and,
            )
            tb = wstage.tile([P, OUT_], dt_u16, tag="tb")
            nc.vector.tensor_tensor(tb[:], t1[:], t2[:], Alu.bitwise_or)
            wt = wpool.tile([P, OUT_], dt_bf16, tag=f"wT{dt}")
            nc.scalar.activation(
                wt[:], tb[:].bitcast(dt_bf16),
                mybir.ActivationFunctionType.Copy, scale=s_eff[:],
            )
            wts.append(wt)

        # ---- main loop ----
        for mb in range(NMB):
            xts = []
            for dt in range(DT):
                xt = xp.tile([P, MB], dt_bf16, tag="xT")
                nc.sync.dma_start(
                    xt[:], x[mb * MB:(mb + 1) * MB, dt * P:(dt + 1) * P],
                    transpose=True,
                )
                xts.append(xt)
            for ms in range(MS):
                psums = [
                    pp.tile([P, NO], dt_f32, tag="ps", name=f"ps{og}")
                    for og in range(OG)
                ]
                for dt in range(DT):
                    lhsT = xts[dt][:, ms * P:(ms + 1) * P]
                    for og in range(OG):
                        nc.tensor.matmul(
                            psums[og][:], lhsT, wts[dt][:, og * NO:(og + 1) * NO],
                            start=(dt == 0), stop=(dt == DT - 1),
                        )
                ot = op.tile([P, OUT_], dt_bf16, tag="ot")
                for og in range(OG):
                    nc.vector.tensor_tensor(
                        ot[:, og * NO:(og + 1) * NO], psums[og][:],
                        bias_t[:, og * NO:(og + 1) * NO], Alu.add,
                    )
                row0 = mb * MB + ms * P
                nc.sync.dma_start(out[row0:row0 + P, :], ot[:])


def build_nc(M=M_CORE, IN_=IN, OUT_=OUT, MB=512, NO=512):
    nc = bacc.Bacc(
        "TRN2", target_bir_lowering=False, debug=False, num_devices=N_CORES
    )
    x_d = nc.dram_tensor("x", [M, IN_], mybir.dt.bfloat16, kind="ExternalInput")
    w_d = nc.dram_tensor("w_t", [IN_, OUT_], mybir.dt.uint8, kind="ExternalInput")
    s_d = nc.dram_tensor("scale", [1, 1], mybir.dt.float32, kind="ExternalInput")
    b_d = nc.dram_tensor("bias", [1, OUT_], mybir.dt.bfloat16, kind="ExternalInput")
    o_d = nc.dram_tensor("out", [M, OUT_], mybir.dt.bfloat16, kind="ExternalOutput")
    with tile.TileContext(nc) as tc:
        emit_fp8linear(
            tc, o_d.ap(), x_d.ap(), w_d.ap(), s_d.ap(), b_d.ap(),
            M, IN_, OUT_, MB, NO,
        )
    nc.compile()
    return nc


_NC_CACHE = {}


def kernel(x, weight_fp8, scale_w, bias):
    assert x.shape == (B, S, IN) and weight_fp8.shape == (OUT, IN)

    if "nc" not in _NC_CACHE:
        _NC_CACHE["nc"] = build_nc()
    nc = _NC_CACHE["nc"]

    x2 = np.ascontiguousarray(x.reshape(M_TOTAL, IN))
    # W^T as raw fp8 bytes, d-major so the contraction dim lands on SBUF
    # partitions without an on-device transpose.
    w_t_u8 = np.ascontiguousarray(weight_fp8.view(np.uint8).T)
    s = np.asarray(scale_w, dtype=np.float32).reshape(1, 1)
    b_row = np.ascontiguousarray(bias.reshape(1, OUT))

    in_maps = [
        {
            "x": x2[c * M_CORE:(c + 1) * M_CORE],
            "w_t": w_t_u8,
            "scale": s,
            "bias": b_row,
        }
        for c in range(N_CORES)
    ]
    res = run_bass_kernel_spmd(nc, in_maps, list(range(N_CORES)))
    shards = [res.results[c]["out"] for c in range(N_CORES)]
    out = np.concatenate(shards, axis=0).reshape(B, S, OUT)
    return out.astype(ml_dtypes.bfloat16, copy=False)


# revision 11
# speedup vs baseline: 1.1973x; 1.1973x over previous
"""FP8Linear Trainium2 kernel.

Computes out = x @ (dequant(weight_fp8) * scale_w)^T + bias for
x: (4, 8192, 2048) bf16, weight_fp8: (2048, 2048) fp8_e4m3fn, scale_w: scalar f32,
bias: (2048,) bf16  ->  out: (4, 8192, 2048) bf16.

Strategy: data-parallel over rows. x flattens to (32768, 2048); each of the 8
NeuronCores gets a contiguous 4096-row slice, the (small, 4MB) weight is
replicated, and each core computes its slice of the output independently — no
collectives. Per core this is compute-bound: ~34 GFLOP of bf16 matmul (~437us
at peak) vs ~38MB of DMA traffic.

fp8 handling: the e4m3fn bytes are shipped as uint8 (the OCP format is NOT
TRN's FP8_EXP4 — values in (240, 448] would decode as NaN/Inf on TRN), and
dequantized on-device with integer ops:
    bf16_bits = ((b << 4) & 0x07F0) | ((b << 8) & 0x8000)
which places exponent e at the bf16 exponent field, i.e. value = fp8_value *
2^-120 exactly (incl. subnormals). One ACT copy with scale = 2^120 *
bf16(scale_w) then produces weights bit-identical to the reference's
  (fp8 -> bf16) * bf16(scale_w).
"""

import sys

sys.path.insert(0, "/opt/trn_rl_repo")

import numpy as np
import ml_dtypes

import concourse.bass as bass
import concourse.mybir as mybir
import concourse.tile as tile
from concourse import bacc
from concourse.bass_utils import run_bass_kernel_spmd

P = 128
N_CORES = 8

# Full problem dims (hardcoded per the contract).
B, S, IN, OUT = 4, 8192, 2048, 2048
M_TOTAL = B * S
M_CORE = M_TOTAL // N_CORES


def emit_fp8linear(tc, out, x, w_t, scale, bias_row, M, IN_, OUT_, MB, NO,
                   opts=None):
    """Emit the per-core program.

    out:   [M, OUT_]  bf16 DRAM (ExternalOutput)
    x:     [M, IN_]   bf16 DRAM
    w_t:   [IN_, OUT_] uint8 DRAM — fp8e4m3fn bytes of W^T (d-major)
    scale: [1, 1]     f32 DRAM
    bias_row: [1, OUT_] bf16 DRAM
    MB: m-block rows (DMA-transpose granularity), NO: matmul moving free dim.
    """
    opts = opts or {}
    decode_gpsimd_every = opts.get("decode_gpsimd_every", 0)  # 0=DVE only
    skip_decode = opts.get("skip_decode", False)  # w_t is pre-decoded bf16
    xt_bufs = opts.get("xt_bufs", None)
    ramp = opts.get("ramp", ())  # leading m-block sizes, e.g. (128, 128, 256)
    split_queues = opts.get("split_queues", True)
    nc = tc.nc
    DT = IN_ // P  # contraction (d) tiles
    OG = OUT_ // NO  # output column groups
    MS = MB // P  # 128-row subtiles per m block
    # m-block row schedule: optional small leading blocks so the PE can start
    # before the full first 512-row transpose lands, then MB-row blocks.
    blocks = list(ramp)
    assert sum(blocks) % MB == 0 or not blocks
    blocks += [MB] * ((M - sum(blocks)) // MB)
    assert sum(blocks) == M
    # second HWDGE queue (ACT sequencer) for weight/bias/scale loads + stores
    dma_w = nc.scalar if split_queues else nc.sync
    dma_x = nc.sync
    dt_bf16 = mybir.dt.bfloat16
    dt_u8 = mybir.dt.uint8
    dt_u16 = mybir.dt.uint16
    dt_f32 = mybir.dt.float32
    Alu = mybir.AluOpType

    with (
        tc.tile_pool(name="const", bufs=1) as const,
        tc.tile_pool(name="wpool", bufs=1) as wpool,
        tc.tile_pool(name="wstage", bufs=2) as wstage,
        tc.tile_pool(name="xT", bufs=xt_bufs or 2 * DT) as xp,
        tc.tile_pool(name="psum", bufs=8, space="PSUM") as pp,
        tc.tile_pool(name="obuf", bufs=4) as op,
    ):
        # ---- constants ----
        bias_t = const.tile([P, OUT_], dt_bf16)
        dma_w.dma_start(bias_t[:], bias_row.to_broadcast((P, OUT_)))

        s_raw = const.tile([P, 1], dt_f32)
        dma_w.dma_start(s_raw[:], scale.to_broadcast((P, 1)))
        # round scale to bf16 (reference multiplies by bf16(scale_w)), then
        # fold in the 2^120 exponent-offset of the integer decode below.
        s_bf = const.tile([P, 1], dt_bf16)
        nc.vector.tensor_copy(s_bf[:], s_raw[:])
        s_eff = const.tile([P, 1], dt_f32)
        nc.vector.tensor_scalar(
            out=s_eff[:], in0=s_bf[:], scalar1=float(2.0**120), scalar2=None,
            op0=Alu.mult,
        )

        # ---- block-0 x transposes first so the PE's lhsT path fills early ----
        xts0 = []
        for dt in range(DT):
            xt = xp.tile([P, blocks[0]], dt_bf16, tag="xT", name=f"xT0_{dt}")
            dma_x.dma_start(
                xt[:], x[0:blocks[0], dt * P:(dt + 1) * P], transpose=True
            )
            xts0.append(xt)

        # ---- weight decode: wT[dt] = bf16 dequant of W^T[dt*P:(dt+1)*P, :] ----
        wts = []
        for dt in range(DT):
            wt = wpool.tile([P, OUT_], dt_bf16, tag=f"wT{dt}", name=f"wT{dt}")
            if skip_decode:
                dma_w.dma_start(wt[:], w_t[dt * P:(dt + 1) * P, :])
                wts.append(wt)
                continue
            wu8 = wstage.tile([P, OUT_], dt_u8, tag="wu8", name=f"wu8_{dt}")
            dma_w.dma_start(wu8[:], w_t[dt * P:(dt + 1) * P, :])
            eng = (
                nc.gpsimd
                if decode_gpsimd_every and dt % decode_gpsimd_every == 0
                else nc.vector
            )
            wu16 = wstage.tile([P, OUT_], dt_u16, tag="wu16", name=f"wu16_{dt}")
            eng.tensor_copy(wu16[:], wu8[:])  # zero-extend u8 -> u16
            t1 = wstage.tile([P, OUT_], dt_u16, tag="t1", name=f"t1_{dt}")
            eng.tensor_scalar(
                out=t1[:], in0=wu16[:], scalar1=4, scalar2=0x07F0,
                op0=Alu.logical_shift_left, op1=Alu.bitwise_and,
            )
            t2 = wstage.tile([P, OUT_], dt_u16, tag="t2", name=f"t2_{dt}")
            eng.tensor_scalar(
                out=t2[:], in0=wu16[:], scalar1=8, scalar2=0x8000,
                op0=Alu.logical_shift_left, op1=Alu.bitwise_and,
            )
            tb = wstage.tile([P, OUT_], dt_u16, tag="tb", name=f"tb_{dt}")
            eng.tensor_tensor(tb[:], t1[:], t2[:], Alu.bitwise_or)
            nc.scalar.activation(
                wt[:], tb[:].bitcast(dt_bf16),
                mybir.ActivationFunctionType.Copy, scale=s_eff[:],
            )
            wts.append(wt)

        # ---- main loop ----
        brow = 0
        for bi, rows_b in enumerate(blocks):
            if bi == 0:
                xts = xts0
            else:
                xts = []
                for dt in range(DT):
                    xt = xp.tile(
                        [P, rows_b], dt_bf16, tag="xT", name=f"xT{bi}_{dt}"
                    )
                    dma_x.dma_start(
                        xt[:], x[brow:brow + rows_b, dt * P:(dt + 1) * P],
                        transpose=True,
                    )
                    xts.append(xt)
            for ms in range(rows_b // P):
                psums = [
                    pp.tile([P, NO], dt_f32, tag="ps", name=f"ps{og}")
                    for og in range(OG)
                ]
                for dt in range(DT):
                    lhsT = xts[dt][:, ms * P:(ms + 1) * P]
                    for og in range(OG):
                        nc.tensor.matmul(
                            psums[og][:], lhsT, wts[dt][:, og * NO:(og + 1) * NO],
                            start=(dt == 0), stop=(dt == DT - 1),
                        )
                ot = op.tile([P, OUT_], dt_bf16, tag="ot")
                for og in range(OG):
                    nc.vector.tensor_tensor(
                        ot[:, og * NO:(og + 1) * NO], psums[og][:],
                        bias_t[:, og * NO:(og + 1) * NO], Alu.add,
                    )
                row0 = brow + ms * P
                dma_w.dma_start(out[row0:row0 + P, :], ot[:])
            brow += rows_b


def build_nc(M=M_CORE, IN_=IN, OUT_=OUT, MB=512, NO=512, opts=None):
    opts = opts or {}
    nc = bacc.Bacc(
        "TRN2", target_bir_lowering=False, debug=False, num_devices=N_CORES
    )
    w_dtype = (
        mybir.dt.bfloat16 if opts.get("skip_decode") else mybir.dt.uint8
    )
    x_d = nc.dram_tensor("x", [M, IN_], mybir.dt.bfloat16, kind="ExternalInput")
    w_d = nc.dram_tensor("w_t", [IN_, OUT_], w_dtype, kind="ExternalInput")
    s_d = nc.dram_tensor("scale", [1, 1], mybir.dt.float32, kind="ExternalInput")
    b_d = nc.dram_tensor("bias", [1, OUT_], mybir.dt.bfloat16, kind="ExternalInput")
    o_d = nc.dram_tensor("out", [M, OUT_], mybir.dt.bfloat16, kind="ExternalOutput")
    with tile.TileContext(nc) as tc:
        emit_fp8linear(
            tc, o_d.ap(), x_d.ap(), w_d.ap(), s_d.ap(), b_d.ap(),
            M, IN_, OUT_, MB, NO, opts=opts,
        )
    nc.compile()
    return nc


_NC_CACHE = {}


def kernel(x, weight_fp8, scale_w, bias):
    assert x.shape == (B, S, IN) and weight_fp8.shape == (OUT, IN)

    if "nc" not in _NC_CACHE:
        _NC_CACHE["nc"] = build_nc()
    nc = _NC_CACHE["nc"]

    x2 = np.ascontiguousarray(x.reshape(M_TOTAL, IN))
    # W^T as raw fp8 bytes, d-major so the contraction dim lands on SBUF
    # partitions without an on-device transpose.
    w_t_u8 = np.ascontiguousarray(weight_fp8.view(np.uint8).T)
    s = np.asarray(scale_w, dtype=np.float32).reshape(1, 1)
    b_row = np.ascontiguousarray(bias.reshape(1, OUT))

    in_maps = [
        {
            "x": x2[c * M_CORE:(c + 1) * M_CORE],
            "w_t": w_t_u8,
            "scale": s,
            "bias": b_row,
        }
        for c in range(N_CORES)
    ]
    res = run_bass_kernel_spmd(nc, in_maps, list(range(N_CORES)))
    shards = [res.results[c]["out"] for c in range(N_CORES)]
    out = np.concatenate(shards, axis=0).reshape(B, S, OUT)
    return out.astype(ml_dtypes.bfloat16, copy=False)
